# revision 33
# baseline (speedup 1.0000x reference)
"""Causal self-attention (B=4, T=2048, C=1024, H=16) on 8 TRN2 NeuronCores.

Sharding: core = (batch, head-group) on a 4x2 grid.  Each core computes the
attention output of 8 heads for one batch element plus its partial out-proj
(y^T = w_out_slice^T @ out_heads^T); the two head-groups of a batch are summed
on the host (the "out_proj all-reduce"), where the final bias is also added.

On-chip dataflow is fully transposed so no transposes are ever needed:
  qk^T  = w_qkv_slice^T @ x^T          (C on partitions)
  v     = x @ w_v_slice                (T on partitions, natural)
  S^T   = k_h @ q_h^T                  (k-positions on partitions)
  P^T   = exp(S^T) (span-limited)      (no max-subtraction: scores ~ N(0,1))
  outT  = [v|1]^T @ P^T                (ones column accumulates sum-of-exp)
  y^T   = w_out_slice^T @ (outT/sumexp)

Attention matmuls run in 64x128 PE-tiled mode: head_dim is 64, so the even
head occupies array rows 0-63 (tile T0) and the odd head rows 64-127 (T8);
the two quadrant instruction streams execute concurrently, doubling score
throughput and halving the PV key-contraction depth per instruction.  The
PV output keeps the ones column (M=65 <= 128), so sum-of-exp stays free.
All full-128 work (qk/v projections of later pairs, out-proj chunks) is
injected between attention blocks as "filler" while the exp-bound softmax
pipeline catches up; PE tiling-mode switches cost ~150ns so fillers are
chunked coarsely.
"""

import sys
import types

if "/opt/trn_rl_repo" not in sys.path:
    sys.path.insert(0, "/opt/trn_rl_repo")

import numpy as np


def _install_ntff_hook_shim():
    """antenv.axon_hooks is missing in this image; provide it so that
    run_bass_kernel_spmd(trace=True) can capture NTFF profiles."""
    if "antenv.axon_hooks" in sys.modules:
        return
    try:
        from trn_agent_boot.trn_boot import _ntff_profile_via_ctypes

        hook = _ntff_profile_via_ctypes("/opt/axon/libaxon_pjrt.so")
    except Exception:
        hook = None
    m = types.ModuleType("antenv.axon_hooks")
    m.get_axon_ntff_profile_hook = lambda: hook
    sys.modules["antenv.axon_hooks"] = m


_install_ntff_hook_shim()

import concourse.bass as bass  # noqa: E402
from concourse import bacc  # noqa: E402
import concourse.mybir as mybir  # noqa: E402
import concourse.tile as tile  # noqa: E402
from concourse.bass_utils import run_bass_kernel_spmd  # noqa: E402

BF16 = mybir.dt.bfloat16
F32 = mybir.dt.float32
NPBF16 = mybir.dt.np(BF16)
EXP = mybir.ActivationFunctionType.Exp

B, T, C = 4, 2048, 1024
H, DH = 16, 64
HC = 8           # heads per core
CK = C // 128    # 8 contraction chunks over C
TB = T // 128    # 16 key blocks / T row blocks
QC = T // 512    # 4 query chunks
SCALE = 1.0 / np.sqrt(DH)

TRACE = False          # set True (e.g. from test.py) to capture an NTFF profile
LAST_RESULT = None     # BassKernelResults of the last run (exec_time_ns etc.)

_CACHE = None


def _build():
    nc = bacc.Bacc("TRN2", target_bir_lowering=False, debug=False, num_devices=8)

    xT = nc.dram_tensor("xT", [C, T], BF16, kind="ExternalInput")
    wqkv = nc.dram_tensor("wqkv", [C, 3 * 512], BF16, kind="ExternalInput")
    bqk = nc.dram_tensor("bqk", [128, CK], F32, kind="ExternalInput")
    bv = nc.dram_tensor("bv", [64, HC], F32, kind="ExternalInput")
    wout = nc.dram_tensor("wout", [512, C], BF16, kind="ExternalInput")
    tri = nc.dram_tensor("tri", [128, 128], BF16, kind="ExternalInput")
    sel = nc.dram_tensor("sel", [128, 128], BF16, kind="ExternalInput")
    yT = nc.dram_tensor("yT", [C, T], BF16, kind="ExternalOutput")

    VROW = HC * 65  # 8 x (64 v-dims + ones column)

    with tile.TileContext(nc) as tc:
        with (
            tc.tile_pool(name="persist", bufs=1) as pp,
            tc.tile_pool(name="sc", bufs=2, space="PSUM") as scp,
            tc.tile_pool(name="oa", bufs=4, space="PSUM") as oap,
            tc.tile_pool(name="pt", bufs=4) as ptp,
            tc.tile_pool(name="nrm", bufs=2) as nrm,
            tc.tile_pool(name="yst", bufs=3) as yst,
        ):
            QT = [pp.tile([128, T], BF16, tag=f"qt{p}", name=f"qt{p}")
                  for p in range(4)]
            KB = [pp.tile([128, T], BF16, tag=f"kb{p}", name=f"kb{p}")
                  for p in range(4)]
            OT = [pp.tile([128, T], BF16, tag=f"ot{p}", name=f"ot{p}")
                  for p in range(4)]
            VA = pp.tile([128, TB, VROW], BF16, tag="va")
            TRI = pp.tile([128, 128], BF16, tag="tri")
            SEL = pp.tile([128, 128], BF16, tag="sel")
            WOUT = pp.tile([128, 4, C], BF16, tag="wout")
            BQK = pp.tile([128, CK], F32, tag="bqk")
            BV = pp.tile([64, HC], F32, tag="bv")
            XT = pp.tile([128, CK, T], BF16, tag="xt")
            WQ = pp.tile([128, CK, 1536], BF16, tag="wq")
            RRP = [pp.tile([128, 512], BF16, tag=f"rrp{i}", name=f"rrp{i}")
                   for i in range(3)]

            # ---- input loads, need-ordered so attention starts ASAP ----
            # group a: x n=0 quarter + p0 q/k weight columns + biases
            # group b: v weight columns + x n=1 quarter
            # group c: everything else (needed only by fillers much later)
            _ldq = [nc.sync, nc.scalar, nc.gpsimd]
            nc.sync.dma_start(BQK[:], bqk[:])
            nc.sync.dma_start(BV[:], bv[:])
            for kc in range(CK):
                _ldq[kc % 3].dma_start(
                    XT[:, kc, 0:512], xT[kc * 128:(kc + 1) * 128, 0:512]
                )
                _ldq[(kc + 1) % 3].dma_start(
                    WQ[:, kc, 0:128], wqkv[kc * 128:(kc + 1) * 128, 0:128]
                )
                _ldq[(kc + 2) % 3].dma_start(
                    WQ[:, kc, 512:640], wqkv[kc * 128:(kc + 1) * 128, 512:640]
                )
            for kc in range(CK):
                _ldq[kc % 3].dma_start(
                    WQ[:, kc, 1024:1536], wqkv[kc * 128:(kc + 1) * 128, 1024:1536]
                )
                _ldq[(kc + 1) % 3].dma_start(
                    XT[:, kc, 512:1024], xT[kc * 128:(kc + 1) * 128, 512:1024]
                )
            nc.gpsimd.dma_start(TRI[:], tri[:])
            nc.gpsimd.dma_start(SEL[:], sel[:])
            for kc in range(CK):
                _ldq[kc % 3].dma_start(
                    WQ[:, kc, 128:512], wqkv[kc * 128:(kc + 1) * 128, 128:512]
                )
                _ldq[(kc + 1) % 3].dma_start(
                    WQ[:, kc, 640:1024], wqkv[kc * 128:(kc + 1) * 128, 640:1024]
                )
                _ldq[(kc + 2) % 3].dma_start(
                    XT[:, kc, 1024:2048], xT[kc * 128:(kc + 1) * 128, 1024:2048]
                )
            for kc in range(4):
                nc.gpsimd.dma_start(WOUT[:, kc, :], wout[kc * 128:(kc + 1) * 128, :])

            nc.vector.memset(VA[:], 1.0)   # ones columns; v copies overwrite
            nc.vector.memset(RRP[0][:], 0.0)
            nc.vector.memset(RRP[1][:], 0.0)

            # ---------------- full-mode building blocks ----------------
            def emit_qk_wave(pair, jobs):
                """kc-outer projection wave: 2-4 (qk, n) jobs at once so the
                PE can chase the x DMA chunk-by-chunk at startup."""
                tiles = [scp.tile([128, 1024], F32, tag="sc", name="qkps")
                         for _ in range((len(jobs) + 1) // 2)]
                psls = [tiles[idx // 2][:, (idx % 2) * 512:(idx % 2 + 1) * 512]
                        for idx in range(len(jobs))]
                for kc in range(CK):
                    for idx, (qk, n) in enumerate(jobs):
                        m = pair + 4 * qk
                        nc.tensor.matmul(
                            psls[idx],
                            WQ[:, kc, m * 128:(m + 1) * 128],
                            XT[:, kc, n * 512:(n + 1) * 512],
                            start=(kc == 0),
                            stop=(kc == CK - 1),
                        )
                for idx, (qk, n) in enumerate(jobs):
                    m = pair + 4 * qk
                    dst = QT[pair] if qk == 0 else KB[pair]
                    nc.vector.tensor_scalar_add(
                        dst[:, n * 512:(n + 1) * 512], psls[idx], BQK[:, m:m + 1]
                    )

            def emit_qk1(pair, qk, n):
                """one projection job (8 matmuls + evac) — filler unit"""
                psl = scp.tile([128, 512], F32, tag="sc", name="qkps")
                m = pair + 4 * qk
                for kc in range(CK):
                    nc.tensor.matmul(
                        psl[:],
                        WQ[:, kc, m * 128:(m + 1) * 128],
                        XT[:, kc, n * 512:(n + 1) * 512],
                        start=(kc == 0),
                        stop=(kc == CK - 1),
                    )
                dst = QT[pair] if qk == 0 else KB[pair]
                nc.vector.tensor_scalar_add(
                    dst[:, n * 512:(n + 1) * 512], psl[:], BQK[:, m:m + 1]
                )

            def emit_v1(t):
                """v projection for one T row block — filler unit"""
                psl = scp.tile([128, 512], F32, tag="sc", name="vps")
                for kc in range(CK):
                    nc.tensor.matmul(
                        psl[:],
                        XT[:, kc, t * 128:(t + 1) * 128],
                        WQ[:, kc, 1024:1536],
                        start=(kc == 0),
                        stop=(kc == CK - 1),
                    )
                src = psl.rearrange("p (h c) -> p h c", c=64)
                dst = VA[:, t, :].rearrange("p (h c) -> p h c", c=65)[:, :, 0:64]
                nc.vector.tensor_copy(dst, src)

            _yq = [0]

            def emit_y1(n, mo, tail=False):
                """out-proj for one row chunk of y^T — filler unit"""
                psl = scp.tile([128, 512], F32, tag="sc", name="yps")
                for kc in range(4):
                    nc.tensor.matmul(
                        psl[:],
                        WOUT[:, kc, mo * 128:(mo + 1) * 128],
                        OT[kc][:, n * 512:(n + 1) * 512],
                        start=(kc == 0),
                        stop=(kc == 3),
                    )
                ys = yst.tile([128, 512], BF16, tag="ys", name="ys")
                nc.vector.tensor_copy(ys[:], psl[:])
                q = _ldq[_yq[0] % 3]
                _yq[0] += 1
                q.dma_start(
                    yT[mo * 128:(mo + 1) * 128, n * 512:(n + 1) * 512], ys[:]
                )

            def emit_v2(t2):
                """v projection for two T row blocks — filler unit"""
                t3 = scp.tile([128, 1024], F32, tag="sc", name="vps")
                for s in range(2):
                    t = t2 + s
                    psl = t3[:, s * 512:(s + 1) * 512]
                    for kc in range(CK):
                        nc.tensor.matmul(
                            psl,
                            XT[:, kc, t * 128:(t + 1) * 128],
                            WQ[:, kc, 1024:1536],
                            start=(kc == 0),
                            stop=(kc == CK - 1),
                        )
                    src = psl.rearrange("p (h c) -> p h c", c=64)
                    dst = VA[:, t, :].rearrange("p (h c) -> p h c", c=65)[:, :, 0:64]
                    nc.vector.tensor_copy(dst, src)

            def emit_y2(n, mo2):
                """out-proj for two row chunks of y^T — filler unit"""
                t3 = scp.tile([128, 1024], F32, tag="sc", name="yps")
                for s in range(2):
                    mo = mo2 + s
                    psl = t3[:, s * 512:(s + 1) * 512]
                    for kc in range(4):
                        nc.tensor.matmul(
                            psl,
                            WOUT[:, kc, mo * 128:(mo + 1) * 128],
                            OT[kc][:, n * 512:(n + 1) * 512],
                            start=(kc == 0),
                            stop=(kc == 3),
                        )
                    ys = yst.tile([128, 512], BF16, tag="ys", name="ys")
                    nc.vector.tensor_copy(ys[:], psl)
                    nc.gpsimd.dma_start(
                        yT[mo * 128:(mo + 1) * 128, n * 512:(n + 1) * 512], ys[:]
                    )

            # ---------------- softmax normalization ----------------
            _nrm_ctr = [0]
            pending_norms = []

            def norm_part1(pair, j, oaccs):
                """Merge the lo/hi key-half accumulators per head, spread the
                sum-of-exp row over 64 partitions via DMA, take reciprocals,
                and gather both heads' 1/sumexp into rows 0/1 of an RRP tile.
                Returns state for the deferred part 2."""
                le, he, ol, oh = oaccs
                ctr = _nrm_ctr[0]
                _nrm_ctr[0] += 1
                rrp = RRP[ctr % 3]
                t1s = []
                for s, (lo_acc, hi_acc) in enumerate(((le, he), (ol, oh))):
                    t1 = nrm.tile([65, 512], F32, tag=f"t1{s}", name=f"t1{s}",
                                  bufs=3)
                    nc.vector.tensor_copy(t1[:], lo_acc[0:65, :])
                    nc.vector.tensor_add(t1[:], t1[:], hi_acc[0:65, :])
                    rs = nrm.tile([64, 8], F32, tag=f"rs{s}", name=f"rs{s}")
                    nc.sync.dma_start(out=rs[:], in_=t1[64:65, :])
                    nc.vector.reciprocal(rs[:], rs[:])
                    rsb = nrm.tile([64, 8], BF16, tag=f"rsb{s}", name=f"rsb{s}")
                    nc.vector.tensor_copy(rsb[:], rs[:])
                    nc.sync.dma_start(out=rrp[s:s + 1, :], in_=rsb[:])
                    t1s.append(t1)
                return pair, j, rrp, t1s

            def norm_part2(state):
                """Deferred: broadcast 1/sumexp to 64 partitions per head with
                one 64-row-tiled selector matmul, scale, add v-bias, store."""
                pair, j, rrp, t1s = state
                bc = scp.tile([128, 1024], F32, tag="sc", name="bc")
                nc.tensor.matmul(bc[:, 0:512], SEL[0:64, :], rrp[0:64, :],
                                 start=True, stop=True)
                for s in range(2):
                    po = s * 64
                    ost = nrm.tile([64, 512], BF16, tag=f"ost{s}", name=f"ost{s}")
                    nc.vector.tensor_mul(
                        ost[:], t1s[s][0:64, :], bc[po:po + 64, 0:512]
                    )
                    # v-bias is folded into the host-side output bias:
                    # sum_h wout_h^T bv_h is a constant vector over T.
                    nc.gpsimd.dma_start(
                        OT[pair][po:po + 64, j * 512:(j + 1) * 512], ost[:]
                    )

            def flush_norms():
                while pending_norms:
                    norm_part2(pending_norms.pop(0))

            # ---------------- attention (64x128 tiled mode) ----------------
            fillers = []

            def pump(k):
                for _ in range(min(k, len(fillers))):
                    fillers.pop(0)()

            def attn(pair, j, inject):
                """inject: dict i -> number of filler chunks to emit after
                block i (full-mode stretch; one mode switch each way)."""
                nb = 4 * (j + 1)
                oaccs = [oap.tile([128, 512], F32, tag="oacc", name=f"oacc{s}")
                         for s in range(4)]
                le, he, ol, oh = oaccs
                co = [65 * 2 * pair, 65 * (2 * pair + 1)]  # v cols even/odd
                js = slice(j * 512, (j + 1) * 512)
                pts = {}

                def emit_scores(i):
                    # diagonal blocks only produce the causally-valid column
                    # span [off, 512); columns below are fully masked.
                    off = max(0, 128 * (i - 4 * j))
                    sc = scp.tile([128, 1024], F32, tag="sc", name="sc")
                    qsl = slice(j * 512 + off, (j + 1) * 512)
                    nc.tensor.matmul(sc[:, off:512],
                                     KB[pair][0:64, i * 128:(i + 1) * 128],
                                     QT[pair][0:64, qsl], start=True, stop=True)
                    nc.tensor.matmul(sc[:, 512 + off:1024],
                                     KB[pair][64:128, i * 128:(i + 1) * 128],
                                     QT[pair][64:128, qsl], start=True, stop=True)
                    pt = ptp.tile([128, 1024], BF16, tag="pt")
                    d = i - 4 * j
                    if d < 0:
                        nc.scalar.activation(pt[:], sc[:], EXP)
                    else:
                        ptv = pt.rearrange("p (s w) -> p s w", s=2)
                        scv = sc.rearrange("p (s w) -> p s w", s=2)
                        nc.scalar.activation(
                            ptv[:, :, off:512], scv[:, :, off:512], EXP
                        )
                        for s in range(2):
                            sl = pt[:, s * 512 + off:s * 512 + off + 128]
                            nc.gpsimd.tensor_mul(sl, sl, TRI[:])
                    pts[i] = pt

                def emit_pv(i):
                    pt = pts.pop(i)
                    off = max(0, 128 * (i - 4 * j))
                    st, sp = (i == 0), (i == nb - 1)
                    sk = off > 0
                    nc.tensor.matmul(le[0:65, off:512],
                                     VA[0:64, i, co[0]:co[0] + 65],
                                     pt[0:64, off:512], start=st, stop=sp,
                                     skip_group_check=sk)
                    nc.tensor.matmul(oh[0:65, off:512],
                                     VA[64:128, i, co[1]:co[1] + 65],
                                     pt[64:128, 512 + off:1024], start=st, stop=sp,
                                     skip_group_check=sk)
                    nc.tensor.matmul(ol[0:65, off:512],
                                     VA[0:64, i, co[1]:co[1] + 65],
                                     pt[0:64, 512 + off:1024], start=st, stop=sp,
                                     skip_group_check=sk)
                    nc.tensor.matmul(he[0:65, off:512],
                                     VA[64:128, i, co[0]:co[0] + 65],
                                     pt[64:128, off:512], start=st, stop=sp,
                                     skip_group_check=sk)

                # 2-deep software pipeline: PV for block i issues two blocks
                # behind its scores, so exp + triangle-mask latency hides
                # behind subsequent score matmuls.  Fillers go last in a
                # block: scores/PV stall on the earliest events (exp of
                # block i-2), fillers on a free PSUM slot.
                for i in range(nb):
                    if i == 2 and len(pending_norms) > 1:
                        norm_part2(pending_norms.pop(0))
                    if i == 4 and pending_norms:
                        flush_norms()
                    emit_scores(i)
                    if i >= 2:
                        emit_pv(i - 2)
                    if i in inject:
                        pump(inject[i])
                emit_pv(nb - 2)
                emit_pv(nb - 1)
                pending_norms.append(norm_part1(pair, j, oaccs))

            # ---------------- schedule ----------------
            # Minimal head: attn(p0, j=0) needs only the (q,n0)/(k,n0)
            # projections and VA blocks 0-3.  Everything else is filler,
            # ordered so each unit completes well before its consumer
            # (VA t4-7 before j=1 PV, n=1..3 projections before j=1..3).
            emit_qk_wave(0, [(0, 0), (1, 0)])
            for t2 in range(0, 4, 2):
                emit_v2(t2)

            for qk in range(2):
                fillers.append(lambda qk=qk: emit_qk1(0, qk, 1))
            for t in range(4, 8):
                fillers.append(lambda t=t: emit_v1(t))
            for n in range(2, 4):
                for qk in range(2):
                    fillers.append(lambda qk=qk, n=n: emit_qk1(0, qk, n))
            for t in range(8, TB):
                fillers.append(lambda t=t: emit_v1(t))

            def queue_qk(pair):
                for n in range(4):
                    for qk in range(2):
                        fillers.append(
                            lambda qk=qk, n=n: emit_qk1(pair, qk, n)
                        )

            # filler injection points: dense early (self-dependencies of
            # p0's later j-chunks need lead time), sparser later
            inj = {j: {i: 1 for i in range(1, 4 * (j + 1))} for j in range(QC)}
            queue_qk(1)
            for j in range(QC):
                attn(0, j, inj[j])
            queue_qk(2)
            queue_qk(3)
            for j in range(QC):
                attn(1, j, inj[j])

            # pairs 2 and 3 interleaved by query chunk: norms of (2, j)
            # flush inside attn(3, j) and vice versa, so out-proj chunks
            # for column n become legal filler across both pairs' (n+1)
            # windows instead of bunching at the very end.
            for j in range(QC):
                if j >= 1:
                    n = j - 1
                    for mo in range(8):
                        fillers.append(lambda n=n, mo=mo: emit_y1(n, mo))
                nb = 4 * (j + 1)
                attn(2, j, {i: 1 for i in range(6, nb)})
                attn(3, j, {i: 1 for i in range(1, nb)})
            while fillers:
                pump(1)
            flush_norms()
            for mo in range(8):
                emit_y1(3, mo, tail=True)

    nc.compile()
    return nc


def kernel(x, w_qkv, b_qkv, w_out, b_out):
    global _CACHE, LAST_RESULT
    x = np.asarray(x, np.float32)
    w_qkv = np.asarray(w_qkv, np.float32)
    b_qkv = np.asarray(b_qkv, np.float32)
    w_out = np.asarray(w_out, np.float32)
    b_out = np.asarray(b_out, np.float32)

    if _CACHE is None:
        _CACHE = _build()
    nc = _CACHE

    r = np.arange(128)
    tri_np = (r[None, :] >= r[:, None]).astype(np.float32).astype(NPBF16)
    sel_np = np.zeros((128, 128), np.float32)
    sel_np[0, 0:64] = 1.0
    sel_np[1, 64:128] = 1.0
    sel_np = sel_np.astype(NPBF16)

    in_maps = []
    for core in range(8):
        b = core // 2
        g = core % 2
        sl = slice(g * 512, (g + 1) * 512)
        wq = w_qkv[:, 0:1024][:, sl] * SCALE
        wk = w_qkv[:, 1024:2048][:, sl]
        wv = w_qkv[:, 2048:3072][:, sl]
        wqkv_c = np.ascontiguousarray(
            np.concatenate([wq, wk, wv], axis=1).astype(NPBF16)
        )
        bq = b_qkv[0:1024][sl] * SCALE
        bk = b_qkv[1024:2048][sl]
        bqk_c = np.ascontiguousarray(
            np.concatenate([bq, bk]).reshape(CK, 128).T.astype(np.float32)
        )
        bv_c = np.ascontiguousarray(
            b_qkv[2048:3072][sl].reshape(HC, 64).T.astype(np.float32)
        )
        in_maps.append(
            {
                "xT": np.ascontiguousarray(x[b].T.astype(NPBF16)),
                "wqkv": wqkv_c,
                "bqk": bqk_c,
                "bv": bv_c,
                "wout": np.ascontiguousarray(w_out[sl, :].astype(NPBF16)),
                "tri": tri_np,
                "sel": sel_np,
            }
        )

    res = run_bass_kernel_spmd(nc, in_maps, core_ids=list(range(8)), trace=TRACE)
    LAST_RESULT = res

    # device computes attention output without the v-bias; its effect on y
    # is the constant vector bv @ w_out, folded into the output bias here.
    yb = b_out + b_qkv[2048:3072] @ w_out
    out = np.empty((B, T, C), np.float32)
    for b in range(B):
        acc = res.results[2 * b]["yT"].astype(np.float32) + res.results[
            2 * b + 1
        ]["yT"].astype(np.float32)
        out[b] = acc.T + yb[None, :]
    return out


# revision 36
# speedup vs baseline: 1.1211x; 1.1211x over previous
"""Causal self-attention (B=4, T=2048, C=1024, H=16) on 8 TRN2 NeuronCores.

Sharding: core = (batch, head-group) on a 4x2 grid.  Each core computes the
attention output of 8 heads for one batch element plus its partial out-proj
(y^T = w_out_slice^T @ out_heads^T); the two head-groups of a batch are summed
on the host (the "out_proj all-reduce"), where the final bias is also added.

On-chip dataflow is fully transposed so no transposes are ever needed:
  qk^T  = w_qkv_slice^T @ x^T          (C on partitions)
  v     = x @ w_v_slice                (T on partitions, natural)
  S^T   = k_h @ q_h^T                  (k-positions on partitions)
  P^T   = exp(S^T) (span-limited)      (no max-subtraction: scores ~ N(0,1))
  outT  = [v|1]^T @ P^T                (ones column accumulates sum-of-exp)
  y^T   = w_out_slice^T @ (outT/sumexp)

Attention matmuls run in 64x128 PE-tiled mode: head_dim is 64, so the even
head occupies array rows 0-63 (tile T0) and the odd head rows 64-127 (T8);
the two quadrant instruction streams execute concurrently, doubling score
throughput and halving the PV key-contraction depth per instruction.  The
PV output keeps the ones column (M=65 <= 128), so sum-of-exp stays free.
All full-128 work (qk/v projections of later pairs, out-proj chunks) is
injected between attention blocks as "filler" while the exp-bound softmax
pipeline catches up; PE tiling-mode switches cost ~150ns so fillers are
chunked coarsely.
"""

import sys
import types

if "/opt/trn_rl_repo" not in sys.path:
    sys.path.insert(0, "/opt/trn_rl_repo")

import numpy as np


def _install_ntff_hook_shim():
    """antenv.axon_hooks is missing in this image; provide it so that
    run_bass_kernel_spmd(trace=True) can capture NTFF profiles."""
    if "antenv.axon_hooks" in sys.modules:
        return
    try:
        from trn_agent_boot.trn_boot import _ntff_profile_via_ctypes

        hook = _ntff_profile_via_ctypes("/opt/axon/libaxon_pjrt.so")
    except Exception:
        hook = None
    m = types.ModuleType("antenv.axon_hooks")
    m.get_axon_ntff_profile_hook = lambda: hook
    sys.modules["antenv.axon_hooks"] = m


_install_ntff_hook_shim()

import concourse.bass as bass  # noqa: E402
from concourse import bacc  # noqa: E402
import concourse.mybir as mybir  # noqa: E402
import concourse.tile as tile  # noqa: E402
from concourse.bass_utils import run_bass_kernel_spmd  # noqa: E402

BF16 = mybir.dt.bfloat16
F32 = mybir.dt.float32
NPBF16 = mybir.dt.np(BF16)
EXP = mybir.ActivationFunctionType.Exp

B, T, C = 4, 2048, 1024
H, DH = 16, 64
HC = 8           # heads per core
CK = C // 128    # 8 contraction chunks over C
TB = T // 128    # 16 key blocks / T row blocks
QC = T // 512    # 4 query chunks
SCALE = 1.0 / np.sqrt(DH)

TRACE = False          # set True (e.g. from test.py) to capture an NTFF profile
LAST_RESULT = None     # BassKernelResults of the last run (exec_time_ns etc.)

_CACHE = None


def _build():
    nc = bacc.Bacc("TRN2", target_bir_lowering=False, debug=False, num_devices=8)

    xT = nc.dram_tensor("xT", [C, T], BF16, kind="ExternalInput")
    wqkv = nc.dram_tensor("wqkv", [C, 3 * 512], BF16, kind="ExternalInput")
    bqk = nc.dram_tensor("bqk", [128, CK], F32, kind="ExternalInput")
    bv = nc.dram_tensor("bv", [64, HC], F32, kind="ExternalInput")
    wout = nc.dram_tensor("wout", [512, C], BF16, kind="ExternalInput")
    tri = nc.dram_tensor("tri", [128, 128], BF16, kind="ExternalInput")
    sel = nc.dram_tensor("sel", [128, 128], BF16, kind="ExternalInput")
    yT = nc.dram_tensor("yT", [C, T], BF16, kind="ExternalOutput")

    VROW = HC * 65  # 8 x (64 v-dims + ones column)

    with tile.TileContext(nc) as tc:
        with (
            tc.tile_pool(name="persist", bufs=1) as pp,
            tc.tile_pool(name="sc", bufs=2, space="PSUM") as scp,
            tc.tile_pool(name="oa", bufs=4, space="PSUM") as oap,
            tc.tile_pool(name="pt", bufs=4) as ptp,
            tc.tile_pool(name="nrm", bufs=2) as nrm,
            tc.tile_pool(name="yst", bufs=3) as yst,
        ):
            QT = [pp.tile([128, T], BF16, tag=f"qt{p}", name=f"qt{p}")
                  for p in range(4)]
            KB = [pp.tile([128, T], BF16, tag=f"kb{p}", name=f"kb{p}")
                  for p in range(4)]
            OT = [pp.tile([128, T], BF16, tag=f"ot{p}", name=f"ot{p}")
                  for p in range(4)]
            VA = pp.tile([128, TB, VROW], BF16, tag="va")
            TRI = pp.tile([128, 128], BF16, tag="tri")
            SEL = pp.tile([128, 128], BF16, tag="sel")
            WOUT = pp.tile([128, 4, C], BF16, tag="wout")
            BQK = pp.tile([128, CK], F32, tag="bqk")
            BV = pp.tile([64, HC], F32, tag="bv")
            XT = pp.tile([128, CK, T], BF16, tag="xt")
            WQ = pp.tile([128, CK, 1536], BF16, tag="wq")
            RRP = [pp.tile([128, 512], BF16, tag=f"rrp{i}", name=f"rrp{i}")
                   for i in range(3)]

            # ---- input loads, need-ordered so attention starts ASAP ----
            # group a: x n=0 quarter + p0 q/k weight columns + biases
            # group b: v weight columns + x n=1 quarter
            # group c: everything else (needed only by fillers much later)
            _ldq = [nc.sync, nc.scalar, nc.gpsimd]
            nc.sync.dma_start(BQK[:], bqk[:])
            nc.sync.dma_start(BV[:], bv[:])
            for kc in range(CK):
                _ldq[kc % 3].dma_start(
                    XT[:, kc, 0:512], xT[kc * 128:(kc + 1) * 128, 0:512]
                )
                _ldq[(kc + 1) % 3].dma_start(
                    WQ[:, kc, 0:128], wqkv[kc * 128:(kc + 1) * 128, 0:128]
                )
                _ldq[(kc + 2) % 3].dma_start(
                    WQ[:, kc, 512:640], wqkv[kc * 128:(kc + 1) * 128, 512:640]
                )
            for kc in range(CK):
                _ldq[kc % 3].dma_start(
                    WQ[:, kc, 1024:1536], wqkv[kc * 128:(kc + 1) * 128, 1024:1536]
                )
                _ldq[(kc + 1) % 3].dma_start(
                    XT[:, kc, 512:1024], xT[kc * 128:(kc + 1) * 128, 512:1024]
                )
            nc.gpsimd.dma_start(TRI[:], tri[:])
            nc.gpsimd.dma_start(SEL[:], sel[:])
            for kc in range(CK):
                _ldq[kc % 3].dma_start(
                    WQ[:, kc, 128:512], wqkv[kc * 128:(kc + 1) * 128, 128:512]
                )
                _ldq[(kc + 1) % 3].dma_start(
                    WQ[:, kc, 640:1024], wqkv[kc * 128:(kc + 1) * 128, 640:1024]
                )
                _ldq[(kc + 2) % 3].dma_start(
                    XT[:, kc, 1024:2048], xT[kc * 128:(kc + 1) * 128, 1024:2048]
                )
            for kc in range(4):
                nc.gpsimd.dma_start(WOUT[:, kc, :], wout[kc * 128:(kc + 1) * 128, :])

            nc.vector.memset(VA[:], 1.0)   # ones columns; v copies overwrite
            nc.vector.memset(RRP[0][:], 0.0)
            nc.vector.memset(RRP[1][:], 0.0)

            # ---------------- full-mode building blocks ----------------
            def emit_qk_wave(pair, jobs):
                """kc-outer projection wave: 2-4 (qk, n) jobs at once so the
                PE can chase the x DMA chunk-by-chunk at startup."""
                tiles = [scp.tile([128, 1024], F32, tag="sc", name="qkps")
                         for _ in range((len(jobs) + 1) // 2)]
                psls = [tiles[idx // 2][:, (idx % 2) * 512:(idx % 2 + 1) * 512]
                        for idx in range(len(jobs))]
                for kc in range(CK):
                    for idx, (qk, n) in enumerate(jobs):
                        m = pair + 4 * qk
                        nc.tensor.matmul(
                            psls[idx],
                            WQ[:, kc, m * 128:(m + 1) * 128],
                            XT[:, kc, n * 512:(n + 1) * 512],
                            start=(kc == 0),
                            stop=(kc == CK - 1),
                        )
                for idx, (qk, n) in enumerate(jobs):
                    m = pair + 4 * qk
                    dst = QT[pair] if qk == 0 else KB[pair]
                    nc.vector.tensor_scalar_add(
                        dst[:, n * 512:(n + 1) * 512], psls[idx], BQK[:, m:m + 1]
                    )

            def emit_qk1(pair, qk, n):
                """one projection job (8 matmuls + evac) — filler unit"""
                psl = scp.tile([128, 512], F32, tag="sc", name="qkps")
                m = pair + 4 * qk
                for kc in range(CK):
                    nc.tensor.matmul(
                        psl[:],
                        WQ[:, kc, m * 128:(m + 1) * 128],
                        XT[:, kc, n * 512:(n + 1) * 512],
                        start=(kc == 0),
                        stop=(kc == CK - 1),
                    )
                dst = QT[pair] if qk == 0 else KB[pair]
                nc.vector.tensor_scalar_add(
                    dst[:, n * 512:(n + 1) * 512], psl[:], BQK[:, m:m + 1]
                )

            def emit_v1(t):
                """v projection for one T row block — filler unit"""
                psl = scp.tile([128, 512], F32, tag="sc", name="vps")
                for kc in range(CK):
                    nc.tensor.matmul(
                        psl[:],
                        XT[:, kc, t * 128:(t + 1) * 128],
                        WQ[:, kc, 1024:1536],
                        start=(kc == 0),
                        stop=(kc == CK - 1),
                    )
                src = psl.rearrange("p (h c) -> p h c", c=64)
                dst = VA[:, t, :].rearrange("p (h c) -> p h c", c=65)[:, :, 0:64]
                nc.vector.tensor_copy(dst, src)

            _yq = [0]

            def emit_y1(n, mo, tail=False):
                """out-proj for one row chunk of y^T — filler unit"""
                psl = scp.tile([128, 512], F32, tag="sc", name="yps")
                for kc in range(4):
                    nc.tensor.matmul(
                        psl[:],
                        WOUT[:, kc, mo * 128:(mo + 1) * 128],
                        OT[kc][:, n * 512:(n + 1) * 512],
                        start=(kc == 0),
                        stop=(kc == 3),
                    )
                ys = yst.tile([128, 512], BF16, tag="ys", name="ys")
                nc.vector.tensor_copy(ys[:], psl[:])
                q = (nc.sync, nc.gpsimd)[_yq[0] % 2]
                _yq[0] += 1
                q.dma_start(
                    yT[mo * 128:(mo + 1) * 128, n * 512:(n + 1) * 512], ys[:]
                )

            def emit_v2(t2):
                """v projection for two T row blocks — filler unit"""
                t3 = scp.tile([128, 1024], F32, tag="sc", name="vps")
                for s in range(2):
                    t = t2 + s
                    psl = t3[:, s * 512:(s + 1) * 512]
                    for kc in range(CK):
                        nc.tensor.matmul(
                            psl,
                            XT[:, kc, t * 128:(t + 1) * 128],
                            WQ[:, kc, 1024:1536],
                            start=(kc == 0),
                            stop=(kc == CK - 1),
                        )
                    src = psl.rearrange("p (h c) -> p h c", c=64)
                    dst = VA[:, t, :].rearrange("p (h c) -> p h c", c=65)[:, :, 0:64]
                    nc.vector.tensor_copy(dst, src)

            def emit_y2(n, mo2):
                """out-proj for two row chunks of y^T — filler unit"""
                t3 = scp.tile([128, 1024], F32, tag="sc", name="yps")
                for s in range(2):
                    mo = mo2 + s
                    psl = t3[:, s * 512:(s + 1) * 512]
                    for kc in range(4):
                        nc.tensor.matmul(
                            psl,
                            WOUT[:, kc, mo * 128:(mo + 1) * 128],
                            OT[kc][:, n * 512:(n + 1) * 512],
                            start=(kc == 0),
                            stop=(kc == 3),
                        )
                    ys = yst.tile([128, 512], BF16, tag="ys", name="ys")
                    nc.vector.tensor_copy(ys[:], psl)
                    nc.gpsimd.dma_start(
                        yT[mo * 128:(mo + 1) * 128, n * 512:(n + 1) * 512], ys[:]
                    )

            # ---------------- softmax normalization ----------------
            _nrm_ctr = [0]
            pending_norms = []

            def norm_part1(pair, j, oaccs):
                """Merge the lo/hi key-half accumulators per head, spread the
                sum-of-exp row over 64 partitions via DMA, take reciprocals,
                and gather both heads' 1/sumexp into rows 0/1 of an RRP tile.
                Returns state for the deferred part 2."""
                le, he, ol, oh = oaccs
                ctr = _nrm_ctr[0]
                _nrm_ctr[0] += 1
                rrp = RRP[ctr % 3]
                t1s = []
                for s, (lo_acc, hi_acc) in enumerate(((le, he), (ol, oh))):
                    t1 = nrm.tile([65, 512], F32, tag=f"t1{s}", name=f"t1{s}",
                                  bufs=3)
                    nc.vector.tensor_copy(t1[:], lo_acc[0:65, :])
                    nc.vector.tensor_add(t1[:], t1[:], hi_acc[0:65, :])
                    rs = nrm.tile([64, 8], F32, tag=f"rs{s}", name=f"rs{s}")
                    nc.sync.dma_start(out=rs[:], in_=t1[64:65, :])
                    nc.vector.reciprocal(rs[:], rs[:])
                    rsb = nrm.tile([64, 8], BF16, tag=f"rsb{s}", name=f"rsb{s}")
                    nc.vector.tensor_copy(rsb[:], rs[:])
                    nc.sync.dma_start(out=rrp[s:s + 1, :], in_=rsb[:])
                    t1s.append(t1)
                return pair, j, rrp, t1s

            def norm_part2(state):
                """Deferred: broadcast 1/sumexp to 64 partitions per head with
                one 64-row-tiled selector matmul, scale, add v-bias, store."""
                pair, j, rrp, t1s = state
                bc = scp.tile([128, 1024], F32, tag="sc", name="bc")
                nc.tensor.matmul(bc[:, 0:512], SEL[0:64, :], rrp[0:64, :],
                                 start=True, stop=True)
                for s in range(2):
                    po = s * 64
                    ost = nrm.tile([64, 512], BF16, tag=f"ost{s}", name=f"ost{s}")
                    nc.vector.tensor_mul(
                        ost[:], t1s[s][0:64, :], bc[po:po + 64, 0:512]
                    )
                    # v-bias is folded into the host-side output bias:
                    # sum_h wout_h^T bv_h is a constant vector over T.
                    nc.gpsimd.dma_start(
                        OT[pair][po:po + 64, j * 512:(j + 1) * 512], ost[:]
                    )

            def flush_norms():
                while pending_norms:
                    norm_part2(pending_norms.pop(0))

            # ---------------- attention (64x128 tiled mode) ----------------
            fillers = []

            def pump(k):
                for _ in range(min(k, len(fillers))):
                    fillers.pop(0)()

            def attn(pair, j, inject):
                """inject: dict i -> number of filler chunks to emit after
                block i (full-mode stretch; one mode switch each way)."""
                nb = 4 * (j + 1)
                oaccs = [oap.tile([128, 512], F32, tag="oacc", name=f"oacc{s}")
                         for s in range(4)]
                le, he, ol, oh = oaccs
                co = [65 * 2 * pair, 65 * (2 * pair + 1)]  # v cols even/odd
                js = slice(j * 512, (j + 1) * 512)
                pts = {}

                def emit_scores(i):
                    # diagonal blocks only produce the causally-valid column
                    # span [off, 512); columns below are fully masked.
                    off = max(0, 128 * (i - 4 * j))
                    sc = scp.tile([128, 1024], F32, tag="sc", name="sc")
                    qsl = slice(j * 512 + off, (j + 1) * 512)
                    nc.tensor.matmul(sc[:, off:512],
                                     KB[pair][0:64, i * 128:(i + 1) * 128],
                                     QT[pair][0:64, qsl], start=True, stop=True)
                    nc.tensor.matmul(sc[:, 512 + off:1024],
                                     KB[pair][64:128, i * 128:(i + 1) * 128],
                                     QT[pair][64:128, qsl], start=True, stop=True)
                    pt = ptp.tile([128, 1024], BF16, tag="pt")
                    d = i - 4 * j
                    if d < 0:
                        nc.scalar.activation(pt[:], sc[:], EXP)
                    else:
                        ptv = pt.rearrange("p (s w) -> p s w", s=2)
                        scv = sc.rearrange("p (s w) -> p s w", s=2)
                        nc.scalar.activation(
                            ptv[:, :, off:512], scv[:, :, off:512], EXP
                        )
                        for s in range(2):
                            sl = pt[:, s * 512 + off:s * 512 + off + 128]
                            nc.gpsimd.tensor_mul(sl, sl, TRI[:])
                    pts[i] = pt

                def emit_pv(i):
                    pt = pts.pop(i)
                    off = max(0, 128 * (i - 4 * j))
                    st, sp = (i == 0), (i == nb - 1)
                    sk = off > 0
                    nc.tensor.matmul(le[0:65, off:512],
                                     VA[0:64, i, co[0]:co[0] + 65],
                                     pt[0:64, off:512], start=st, stop=sp,
                                     skip_group_check=sk)
                    nc.tensor.matmul(oh[0:65, off:512],
                                     VA[64:128, i, co[1]:co[1] + 65],
                                     pt[64:128, 512 + off:1024], start=st, stop=sp,
                                     skip_group_check=sk)
                    nc.tensor.matmul(ol[0:65, off:512],
                                     VA[0:64, i, co[1]:co[1] + 65],
                                     pt[0:64, 512 + off:1024], start=st, stop=sp,
                                     skip_group_check=sk)
                    nc.tensor.matmul(he[0:65, off:512],
                                     VA[64:128, i, co[0]:co[0] + 65],
                                     pt[64:128, off:512], start=st, stop=sp,
                                     skip_group_check=sk)

                # 2-deep software pipeline: PV for block i issues two blocks
                # behind its scores, so exp + triangle-mask latency hides
                # behind subsequent score matmuls.  Fillers go last in a
                # block: scores/PV stall on the earliest events (exp of
                # block i-2), fillers on a free PSUM slot.
                for i in range(nb):
                    if i == 2 and len(pending_norms) > 1:
                        norm_part2(pending_norms.pop(0))
                    if i == 4 and pending_norms:
                        flush_norms()
                    emit_scores(i)
                    if i >= 2:
                        emit_pv(i - 2)
                    if i in inject:
                        pump(inject[i])
                emit_pv(nb - 2)
                emit_pv(nb - 1)
                pending_norms.append(norm_part1(pair, j, oaccs))

            # ---------------- schedule ----------------
            # Minimal head: attn(p0, j=0) needs only the (q,n0)/(k,n0)
            # projections and VA blocks 0-3.  Everything else is filler,
            # ordered so each unit completes well before its consumer
            # (VA t4-7 before j=1 PV, n=1..3 projections before j=1..3).
            emit_qk_wave(0, [(0, 0), (1, 0)])
            for t2 in range(0, 4, 2):
                emit_v2(t2)

            for qk in range(2):
                fillers.append(lambda qk=qk: emit_qk1(0, qk, 1))
            for t in range(4, 8):
                fillers.append(lambda t=t: emit_v1(t))
            for n in range(2, 4):
                for qk in range(2):
                    fillers.append(lambda qk=qk, n=n: emit_qk1(0, qk, n))
            for t in range(8, TB):
                fillers.append(lambda t=t: emit_v1(t))

            def queue_qk(pair):
                for n in range(4):
                    for qk in range(2):
                        fillers.append(
                            lambda qk=qk, n=n: emit_qk1(pair, qk, n)
                        )

            # filler injection points: dense early (self-dependencies of
            # p0's later j-chunks need lead time), sparser later
            inj = {0: {i: 1 for i in range(1, 4)},
                   1: {i: 1 for i in range(1, 8)},
                   2: {i: 1 for i in range(1, 12, 2)},
                   3: {i: 1 for i in range(1, 16, 2)}}
            queue_qk(1)
            for j in range(QC):
                attn(0, j, inj[j])
            queue_qk(2)
            queue_qk(3)
            for j in range(QC):
                attn(1, j, inj[j])

            # pairs 2 and 3 interleaved by query chunk: norms of (2, j)
            # flush inside attn(3, j) and vice versa, so out-proj chunks
            # for column n become legal filler across both pairs' (n+1)
            # windows instead of bunching at the very end.
            for j in range(QC):
                if j >= 1:
                    n = j - 1
                    for mo in range(8):
                        fillers.append(lambda n=n, mo=mo: emit_y1(n, mo))
                nb = 4 * (j + 1)
                attn(2, j, {i: 1 for i in range(6, nb, 2)})
                attn(3, j, {i: 1 for i in range(1, nb, 2)})
            while fillers:
                pump(1)
            flush_norms()
            for mo in range(8):
                emit_y1(3, mo, tail=True)

    nc.compile()
    return nc


def kernel(x, w_qkv, b_qkv, w_out, b_out):
    global _CACHE, LAST_RESULT
    x = np.asarray(x, np.float32)
    w_qkv = np.asarray(w_qkv, np.float32)
    b_qkv = np.asarray(b_qkv, np.float32)
    w_out = np.asarray(w_out, np.float32)
    b_out = np.asarray(b_out, np.float32)

    if _CACHE is None:
        _CACHE = _build()
    nc = _CACHE

    r = np.arange(128)
    tri_np = (r[None, :] >= r[:, None]).astype(np.float32).astype(NPBF16)
    sel_np = np.zeros((128, 128), np.float32)
    sel_np[0, 0:64] = 1.0
    sel_np[1, 64:128] = 1.0
    sel_np = sel_np.astype(NPBF16)

    in_maps = []
    for core in range(8):
        b = core // 2
        g = core % 2
        sl = slice(g * 512, (g + 1) * 512)
        wq = w_qkv[:, 0:1024][:, sl] * SCALE
        wk = w_qkv[:, 1024:2048][:, sl]
        wv = w_qkv[:, 2048:3072][:, sl]
        wqkv_c = np.ascontiguousarray(
            np.concatenate([wq, wk, wv], axis=1).astype(NPBF16)
        )
        bq = b_qkv[0:1024][sl] * SCALE
        bk = b_qkv[1024:2048][sl]
        bqk_c = np.ascontiguousarray(
            np.concatenate([bq, bk]).reshape(CK, 128).T.astype(np.float32)
        )
        bv_c = np.ascontiguousarray(
            b_qkv[2048:3072][sl].reshape(HC, 64).T.astype(np.float32)
        )
        in_maps.append(
            {
                "xT": np.ascontiguousarray(x[b].T.astype(NPBF16)),
                "wqkv": wqkv_c,
                "bqk": bqk_c,
                "bv": bv_c,
                "wout": np.ascontiguousarray(w_out[sl, :].astype(NPBF16)),
                "tri": tri_np,
                "sel": sel_np,
            }
        )

    res = run_bass_kernel_spmd(nc, in_maps, core_ids=list(range(8)), trace=TRACE)
    LAST_RESULT = res

    # device computes attention output without the v-bias; its effect on y
    # is the constant vector bv @ w_out, folded into the output bias here.
    yb = b_out + b_qkv[2048:3072] @ w_out
    out = np.empty((B, T, C), np.float32)
    for b in range(B):
        acc = res.results[2 * b]["yT"].astype(np.float32) + res.results[
            2 * b + 1
        ]["yT"].astype(np.float32)
        out[b] = acc.T + yb[None, :]
    return out
